# revision 3
# baseline (speedup 1.0000x reference)
"""Diagonal-MVN NLL loss (CNPs loss) on 8 Trainium2 NeuronCores.

loss = -mean_b logprob_b with
  logprob_b = -0.5 * sum_d( log(2pi) + log(var) + (t - mu)^2 / var )
  var       = softplus(log_sigma) = ln(1 + e^ls)

which reduces to a single global sum:
  loss = 0.5*D*log(2pi) + (0.5/B) * sum_{b,d}[ ln(var) + (t-mu)^2 / var ]

Data-parallel over the batch dim: 16384 rows -> 2048 rows per core. Each core
writes per-partition partial sums (two [128, CHUNKS] f32 stats tensors); the
host does the final reduction in float64.

Raw-bass implementation (manual semaphores, max one wait per instruction —
this container's walrus codegen rejects multi-wait instructions and the
custom-DVE ISA ops):
  ScalarE: e = Exp(ls); sp = Ln(e + 1); lv = Ln(sp) [accum_out -> row sums];
           r = Exp(-lv) = 1/var              (single LUT table set)
  VectorE: d = tv - mu; d2 = d*d; q = d2*r; reduce_add(q) -> row sums
"""

import contextlib

import numpy as np

import concourse.bass as bass
from concourse import mybir
from concourse.bass_utils import run_bass_kernel_spmd

LOG_2PI = float(np.log(2.0 * np.pi))

N_CORES = 8
B, TWO_D = 16384, 1024
D = TWO_D // 2            # 512
RPC = B // N_CORES        # rows per core = 2048
P = 128                   # SBUF partitions
CHUNKS = 4                # row-chunks per core
QR = RPC // (CHUNKS * P)  # 128-row groups per chunk = 4
CF = QR * D               # free dim per chunk = 2048
NBUF = 2                  # double buffering

_prog_cache = {}
last_results = None  # BassKernelResults of the most recent run (for profiling)


def _build_program() -> bass.Bass:
    nc = bass.Bass("TRN2", target_bir_lowering=False, debug=False)
    f32 = mybir.dt.float32
    A = mybir.ActivationFunctionType
    Op = mybir.AluOpType

    mu = nc.dram_tensor("mu", [RPC, D], f32, kind="ExternalInput")
    ls = nc.dram_tensor("ls", [RPC, D], f32, kind="ExternalInput")
    tv = nc.dram_tensor("tv", [RPC, D], f32, kind="ExternalInput")
    stats_a = nc.dram_tensor("stats_a", [P, CHUNKS], f32, kind="ExternalOutput")
    stats_v = nc.dram_tensor("stats_v", [P, CHUNKS], f32, kind="ExternalOutput")

    # rows = c*(QR*P) + q*P + p  ->  chunk c, partition p, column block q
    mu_r = mu[:, :].rearrange("(c q p) f -> c p q f", c=CHUNKS, q=QR, p=P)
    ls_r = ls[:, :].rearrange("(c q p) f -> c p q f", c=CHUNKS, q=QR, p=P)
    tv_r = tv[:, :].rearrange("(c q p) f -> c p q f", c=CHUNKS, q=QR, p=P)

    with contextlib.ExitStack() as ctx:
        def sb(name):
            return ctx.enter_context(nc.sbuf_tensor(name, [P, CF], f32))

        def bufs(name):
            return [sb(f"{name}{i}") for i in range(NBUF)]

        ls_b = bufs("ls_t")
        mu_b = bufs("mu_t")
        tv_b = bufs("tv_t")
        e_b = bufs("e_t")
        sp_b = bufs("sp_t")
        lv_b = bufs("lv_t")
        r_b = bufs("r_t")
        d_b = bufs("d_t")
        d2_b = bufs("d2_t")
        q_b = bufs("q_t")
        st_a = ctx.enter_context(nc.sbuf_tensor("st_a", [P, CHUNKS], f32))
        st_v = ctx.enter_context(nc.sbuf_tensor("st_v", [P, CHUNKS], f32))

        sem_ls = ctx.enter_context(nc.semaphore("dma_ls"))
        sem_mt = ctx.enter_context(nc.semaphore("dma_mt"))
        sem_act = ctx.enter_context(nc.semaphore("act"))
        sem_dve = ctx.enter_context(nc.semaphore("dve"))
        sem_out = ctx.enter_context(nc.semaphore("dma_out"))
        block = ctx.enter_context(nc.Block())

        # ACT op k of chunk c increments sem_act to 4c+k+1 (k=0..3:
        # exp, ln1, ln2, r). DVE likewise (sub, mul, qmul, reduce).

        @block.sync
        def _(sync):
            for c in range(CHUNKS):
                b = c % NBUF
                if c >= NBUF:
                    # ls buffer is free once chunk c-NBUF's Exp has read it
                    sync.wait_ge(sem_act, 4 * (c - NBUF) + 1)
                sync.dma_start(ls_b[b][:], ls_r[c]).then_inc(sem_ls, 16)
                if c >= NBUF:
                    # mu/tv buffers free once chunk c-NBUF's sub has read them
                    sync.wait_ge(sem_dve, 4 * (c - NBUF) + 1)
                sync.dma_start(mu_b[b][:], mu_r[c]).then_inc(sem_mt, 16)
                sync.dma_start(tv_b[b][:], tv_r[c]).then_inc(sem_mt, 16)
            sync.wait_ge(sem_act, 4 * CHUNKS)
            sync.dma_start(stats_a[:, :], st_a[:]).then_inc(sem_out, 16)
            sync.wait_ge(sem_dve, 4 * CHUNKS)
            sync.dma_start(stats_v[:, :], st_v[:]).then_inc(sem_out, 16)

        @block.scalar
        def _(scalar):
            for c in range(CHUNKS):
                b = c % NBUF
                scalar.wait_ge(sem_ls, 16 * (c + 1))
                scalar.activation(e_b[b][:], ls_b[b][:], A.Exp).then_inc(sem_act, 1)
                scalar.activation(
                    sp_b[b][:], e_b[b][:], A.Ln, bias=1.0
                ).then_inc(sem_act, 1)
                scalar.activation(
                    lv_b[b][:], sp_b[b][:], A.Ln, accum_out=st_a[:, c : c + 1]
                ).then_inc(sem_act, 1)
                if c >= NBUF:
                    # r buffer free once chunk c-NBUF's q-mul has read it
                    scalar.wait_ge(sem_dve, 4 * (c - NBUF) + 3)
                scalar.activation(
                    r_b[b][:], lv_b[b][:], A.Exp, scale=-1.0
                ).then_inc(sem_act, 1)

        @block.vector
        def _(vector):
            for c in range(CHUNKS):
                b = c % NBUF
                vector.wait_ge(sem_mt, 32 * (c + 1))
                vector.tensor_sub(d_b[b][:], tv_b[b][:], mu_b[b][:]).then_inc(
                    sem_dve, 1
                )
                vector.tensor_mul(d2_b[b][:], d_b[b][:], d_b[b][:]).then_inc(
                    sem_dve, 1
                )
                vector.wait_ge(sem_act, 4 * c + 4)
                vector.tensor_mul(q_b[b][:], d2_b[b][:], r_b[b][:]).then_inc(
                    sem_dve, 1
                )
                vector.tensor_reduce(
                    st_v[:, c : c + 1], q_b[b][:], axis=mybir.AxisListType.X, op=Op.add
                ).then_inc(sem_dve, 1)

    return nc


def _get_program() -> bass.Bass:
    if "nc" not in _prog_cache:
        _prog_cache["nc"] = _build_program()
    return _prog_cache["nc"]


def kernel(outputs: np.ndarray, targets: np.ndarray, **run_kwargs) -> np.ndarray:
    global last_results
    assert outputs.shape == (B, TWO_D) and targets.shape == (B, TWO_D)

    outputs = np.asarray(outputs, dtype=np.float32)
    targets = np.asarray(targets, dtype=np.float32)

    in_maps = []
    for i in range(N_CORES):
        rows = slice(i * RPC, (i + 1) * RPC)
        in_maps.append(
            {
                "mu": np.ascontiguousarray(outputs[rows, :D]),
                "ls": np.ascontiguousarray(outputs[rows, D:]),
                "tv": np.ascontiguousarray(targets[rows, :D]),
            }
        )

    nc = _get_program()
    res = run_bass_kernel_spmd(nc, in_maps, core_ids=list(range(N_CORES)), **run_kwargs)
    last_results = res

    total = 0.0
    for core_out in res.results:
        total += core_out["stats_a"].astype(np.float64).sum()
        total += core_out["stats_v"].astype(np.float64).sum()

    loss = 0.5 * D * LOG_2PI + 0.5 * total / B
    return np.asarray(loss, dtype=np.float32)


if __name__ == "__main__":
    rng = np.random.default_rng(0)
    o = rng.standard_normal((B, TWO_D), dtype=np.float32)
    t = rng.standard_normal((B, TWO_D), dtype=np.float32)
    got = kernel(o, t)
    m, lsg = o[:, :D].astype(np.float64), o[:, D:].astype(np.float64)
    tvv = t[:, :D].astype(np.float64)
    var = np.log1p(np.exp(lsg))
    want = 0.5 * D * LOG_2PI + 0.5 * np.mean(
        np.sum(np.log(var) + (tvv - m) ** 2 / var, axis=1)
    )
    print("got", got, "want", want, "rel", abs(got - want) / abs(want))


# revision 11
# speedup vs baseline: 1.1121x; 1.1121x over previous
"""Diagonal-MVN NLL loss (CNPs loss) on 8 Trainium2 NeuronCores.

loss = -mean_b logprob_b with
  logprob_b = -0.5 * sum_d( log(2pi) + log(var) + (t - mu)^2 / var )
  var       = softplus(log_sigma) = ln(1 + e^ls)

which reduces to a single global sum:
  loss = 0.5*D*log(2pi) + (0.5/B) * sum_{b,d}[ ln(var) + (t-mu)^2 / var ]

Data-parallel over the batch dim: 16384 rows -> 2048 rows per core. The host
pre-packs each core's shard to bf16 in a per-partition-contiguous [128, 8192]
layout (row p = the 16 batch rows p, p+128, ... concatenated), so every DMA
is 128 contiguous descriptors. Each core returns small partial-sum tensors;
the host does the final reduction in float64.

Raw-bass implementation (manual semaphores, max one wait condition per
instruction — this container's walrus rejects multi-wait instructions and
custom-DVE ISA ops). Per [128, 2048] chunk (4 chunks):

  ScalarE phase 1: sp_c = Softplus(ls_c)            (softplus table set)
  ScalarE phase 2: lv = Ln(sp_c) [accum -> row sums of ln(var)]
                   r_c = Exp(-lv) = 1/var  (bf16)   (ln/exp table set)
  VectorE:  d = tv - mu; d2 = d*d; q_c = d2*r_c     (all bf16, 2x mode)
  TensorE:  psum[1,512] += ones[128,1].T @ q_c[:, j*512:...]  (row sums)

Table sets are phase-batched (all softplus first) so only 2 table loads
happen; a scale=0 dummy activation prefetches the softplus set during the
DMA ramp. GpSimd only memsets the ones vector.
"""

import contextlib

import ml_dtypes
import numpy as np

import concourse.bass as bass
from concourse import mybir
from concourse.bass_utils import run_bass_kernel_spmd

LOG_2PI = float(np.log(2.0 * np.pi))
BF16 = ml_dtypes.bfloat16

N_CORES = 8
B, TWO_D = 16384, 1024
D = TWO_D // 2            # 512
RPC = B // N_CORES        # rows per core = 2048
P = 128                   # SBUF partitions
RG = RPC // P             # row-groups per core = 16
FTOT = RG * D             # total free dim per core = 8192
CHUNKS = 4
CF = FTOT // CHUNKS       # free dim per chunk = 2048
NMM = CF // 512           # matmuls per chunk = 4

_prog_cache = {}
last_results = None  # BassKernelResults of the most recent run (for profiling)


def _build_program() -> bass.Bass:
    nc = bass.Bass("TRN2", target_bir_lowering=False, debug=False)
    f32 = mybir.dt.float32
    bf16 = mybir.dt.bfloat16
    A = mybir.ActivationFunctionType
    Op = mybir.AluOpType

    mu = nc.dram_tensor("mu", [P, FTOT], bf16, kind="ExternalInput")
    ls = nc.dram_tensor("ls", [P, FTOT], bf16, kind="ExternalInput")
    tv = nc.dram_tensor("tv", [P, FTOT], bf16, kind="ExternalInput")
    stats_a = nc.dram_tensor("stats_a", [P, CHUNKS], f32, kind="ExternalOutput")
    stats_q = nc.dram_tensor("stats_q", [1, 512], f32, kind="ExternalOutput")

    with contextlib.ExitStack() as ctx:
        def sbuf(name, shape, dt):
            return ctx.enter_context(nc.sbuf_tensor(name, shape, dt))

        ls_t = sbuf("ls_t", [P, FTOT], bf16)
        mu_t = sbuf("mu_t", [P, FTOT], bf16)
        tv_t = sbuf("tv_t", [P, FTOT], bf16)
        e_t = sbuf("e_t", [P, CF], f32)         # per-chunk scratch (ACT only)
        sp_t = sbuf("sp_t", [P, CF], f32)       # per-chunk scratch (ACT only)
        lv_t = sbuf("lv_t", [P, CF], f32)       # per-chunk scratch (ACT only)
        r_b = [sbuf(f"r_t{i}", [P, CF], bf16) for i in range(2)]
        d_t = sbuf("d_t", [P, CF], bf16)        # per-chunk scratch (DVE only)
        d2_t = sbuf("d2_t", [P, CF], bf16)
        q_b = [sbuf(f"q_t{i}", [P, CF], bf16) for i in range(2)]
        st_a = sbuf("st_a", [P, CHUNKS], f32)
        sq_t = sbuf("sq_t", [1, 512], f32)
        ones_t = sbuf("ones_t", [P, 1], bf16)
        dummy = sbuf("dummy_t", [P, 1], f32)

        psum = ctx.enter_context(nc.psum_tensor("acc", [1, 512], f32))

        sem_ls = [ctx.enter_context(nc.semaphore(f"ls{c}")) for c in range(CHUNKS)]
        sem_mt = [ctx.enter_context(nc.semaphore(f"mt{c}")) for c in range(CHUNKS)]
        sem_act = ctx.enter_context(nc.semaphore("act"))
        sem_dve = ctx.enter_context(nc.semaphore("dve"))
        sem_pe = ctx.enter_context(nc.semaphore("pe"))
        sem_ones = ctx.enter_context(nc.semaphore("ones"))
        sem_out = ctx.enter_context(nc.semaphore("out"))
        block = ctx.enter_context(nc.Block())

        # ACT increments: dummy=1; exp_c=4c+2, ln1_c=4c+3, ln2_c=4c+4,
        # r_c=4c+5 (last ln2 = 16, copy = 18).
        # DVE increments per chunk: sub=3c+1, mul=3c+2, qmul=3c+3.
        # PE increments: 4 per chunk -> 4c+j+1.

        def cs(c):  # chunk slice in the [P, FTOT] tensors
            return slice(c * CF, (c + 1) * CF)

        @block.sync
        def _(sync):
            for c in range(CHUNKS):
                sync.dma_start(ls_t[:, cs(c)], ls[:, cs(c)]).then_inc(sem_ls[c], 16)
            for c in range(CHUNKS):
                sync.dma_start(mu_t[:, cs(c)], mu[:, cs(c)]).then_inc(sem_mt[c], 16)
                sync.dma_start(tv_t[:, cs(c)], tv[:, cs(c)]).then_inc(sem_mt[c], 16)
            sync.wait_ge(sem_act, 16)  # last Ln done -> st_a complete
            sync.dma_start(stats_a[:, :], st_a[:]).then_inc(sem_out, 16)
            sync.wait_ge(sem_act, 18)  # psum->sbuf copy done
            sync.dma_start(stats_q[:, :], sq_t[:]).then_inc(sem_out, 16)

        @block.gpsimd
        def _(gpsimd):
            gpsimd.memset(ones_t[:], 1.0)
            gpsimd.nop().then_inc(sem_ones, 1)

        @block.scalar
        def _(scalar):
            # Prefetch the ln/exp table set during the DMA ramp. scale=0
            # means the (garbage) input is never read: exp(0) = 1.
            scalar.activation(dummy[:], dummy[:], A.Exp, scale=0.0).then_inc(
                sem_act, 1
            )
            for c in range(CHUNKS):
                scalar.wait_ge(sem_ls[c], 16)
                scalar.activation(e_t[:], ls_t[:, cs(c)], A.Exp).then_inc(sem_act, 1)
                scalar.activation(sp_t[:], e_t[:], A.Ln, bias=1.0).then_inc(sem_act, 1)
                scalar.activation(
                    lv_t[:], sp_t[:], A.Ln, accum_out=st_a[:, c : c + 1]
                ).then_inc(sem_act, 1)
                if c >= 2:
                    # r buffer free once chunk c-2's q-mul has read it
                    scalar.wait_ge(sem_dve, 3 * (c - 2) + 3)
                scalar.activation(
                    r_b[c % 2][:], lv_t[:], A.Exp, scale=-1.0
                ).then_inc(sem_act, 1)
            scalar.wait_ge(sem_pe, 4 * CHUNKS)
            scalar.copy(sq_t[:], psum[:]).then_inc(sem_act, 1)

        @block.vector
        def _(vector):
            for c in range(CHUNKS):
                vector.wait_ge(sem_mt[c], 32)
                vector.tensor_sub(d_t[:], tv_t[:, cs(c)], mu_t[:, cs(c)]).then_inc(
                    sem_dve, 1
                )
                vector.tensor_mul(d2_t[:], d_t[:], d_t[:]).then_inc(sem_dve, 1)
                vector.wait_ge(sem_act, 4 * c + 5)  # r_c ready
                if c >= 2:
                    # q buffer free once chunk c-2's matmuls have read it
                    vector.wait_ge(sem_pe, 4 * (c - 2) + 4)
                vector.tensor_mul(q_b[c % 2][:], d2_t[:], r_b[c % 2][:]).then_inc(
                    sem_dve, 1
                )

        @block.tensor
        def _(tensor):
            tensor.wait_ge(sem_ones, 1)
            n = CHUNKS * NMM
            k = 0
            for c in range(CHUNKS):
                tensor.wait_ge(sem_dve, 3 * c + 3)  # q_c ready
                for j in range(NMM):
                    nc.tensor.matmul(
                        psum[:, :],
                        ones_t[:],
                        q_b[c % 2][:, j * 512 : (j + 1) * 512],
                        start=(k == 0),
                        stop=(k == n - 1),
                    ).then_inc(sem_pe, 1)
                    k += 1

    return nc


def _get_program() -> bass.Bass:
    if "nc" not in _prog_cache:
        _prog_cache["nc"] = _build_program()
    return _prog_cache["nc"]


def _pack(x: np.ndarray) -> np.ndarray:
    # [2048, 512] -> [128, 8192]: partition p holds rows p, p+128, ...
    return np.ascontiguousarray(
        x.reshape(RG, P, D).transpose(1, 0, 2).reshape(P, FTOT).astype(BF16)
    )


def kernel(outputs: np.ndarray, targets: np.ndarray, **run_kwargs) -> np.ndarray:
    global last_results
    assert outputs.shape == (B, TWO_D) and targets.shape == (B, TWO_D)

    outputs = np.asarray(outputs, dtype=np.float32)
    targets = np.asarray(targets, dtype=np.float32)

    in_maps = []
    for i in range(N_CORES):
        rows = slice(i * RPC, (i + 1) * RPC)
        in_maps.append(
            {
                "mu": _pack(outputs[rows, :D]),
                "ls": _pack(outputs[rows, D:]),
                "tv": _pack(targets[rows, :D]),
            }
        )

    nc = _get_program()
    res = run_bass_kernel_spmd(nc, in_maps, core_ids=list(range(N_CORES)), **run_kwargs)
    last_results = res

    total = 0.0
    for core_out in res.results:
        total += core_out["stats_a"].astype(np.float64).sum()
        total += core_out["stats_q"].astype(np.float64).sum()

    loss = 0.5 * D * LOG_2PI + 0.5 * total / B
    return np.asarray(loss, dtype=np.float32)


if __name__ == "__main__":
    rng = np.random.default_rng(0)
    o = rng.standard_normal((B, TWO_D), dtype=np.float32)
    t = rng.standard_normal((B, TWO_D), dtype=np.float32)
    got = kernel(o, t)
    m, lsg = o[:, :D].astype(np.float64), o[:, D:].astype(np.float64)
    tvv = t[:, :D].astype(np.float64)
    var = np.log1p(np.exp(lsg))
    want = 0.5 * D * LOG_2PI + 0.5 * np.mean(
        np.sum(np.log(var) + (tvv - m) ** 2 / var, axis=1)
    )
    print("got", got, "want", want, "rel", abs(got - want) / abs(want))


# revision 17
# speedup vs baseline: 1.1643x; 1.0469x over previous
"""Diagonal-MVN NLL loss (CNPs loss) on 8 Trainium2 NeuronCores.

loss = -mean_b logprob_b with
  logprob_b = -0.5 * sum_d( log(2pi) + log(var) + (t - mu)^2 / var )
  var       = softplus(log_sigma) = ln(1 + e^ls)

which reduces to a single global sum:
  loss = 0.5*D*log(2pi) + (0.5/B) * sum_{b,d}[ ln(var) + (t-mu)^2 / var ]

Data-parallel over the batch dim: 16384 rows -> 2048 rows per core. The host
pre-packs each core's shard to bf16 in a per-partition-contiguous [128, 8192]
layout (row p = the 16 batch rows p, p+128, ... concatenated), so every DMA
is 128 contiguous descriptors. Each core returns small partial-sum tensors;
the host does the final reduction in float64.

Raw-bass implementation (manual semaphores, max one wait condition per
instruction — this container's walrus rejects multi-wait instructions and
custom-DVE ISA ops). Per [128, 2048] chunk (4 chunks):

  ScalarE phase 1: sp_c = Softplus(ls_c)            (softplus table set)
  ScalarE phase 2: lv = Ln(sp_c) [accum -> row sums of ln(var)]
                   r_c = Exp(-lv) = 1/var  (bf16)   (ln/exp table set)
  VectorE:  d = tv - mu; d2 = d*d; q_c = d2*r_c     (all bf16, 2x mode)
  TensorE:  psum[1,512] += ones[128,1].T @ q_c[:, j*512:...]  (row sums)

Table sets are phase-batched (all softplus first) so only 2 table loads
happen; a scale=0 dummy activation prefetches the softplus set during the
DMA ramp. GpSimd only memsets the ones vector.
"""

import contextlib

import ml_dtypes
import numpy as np

import concourse.bass as bass
from concourse import mybir
from concourse.bass_utils import run_bass_kernel_spmd

LOG_2PI = float(np.log(2.0 * np.pi))
BF16 = ml_dtypes.bfloat16

N_CORES = 8
B, TWO_D = 16384, 1024
D = TWO_D // 2            # 512
RPC = B // N_CORES        # rows per core = 2048
P = 128                   # SBUF partitions
RG = RPC // P             # row-groups per core = 16
FTOT = RG * D             # total free dim per core = 8192
CHUNKS = 4
CF = FTOT // CHUNKS       # free dim per chunk = 2048
NMM = CF // 512           # matmuls per chunk = 4

_prog_cache = {}
last_results = None  # BassKernelResults of the most recent run (for profiling)


def _build_program() -> bass.Bass:
    nc = bass.Bass("TRN2", target_bir_lowering=False, debug=False)
    f32 = mybir.dt.float32
    bf16 = mybir.dt.bfloat16
    A = mybir.ActivationFunctionType
    Op = mybir.AluOpType

    mu = nc.dram_tensor("mu", [P, FTOT], bf16, kind="ExternalInput")
    ls = nc.dram_tensor("ls", [P, FTOT], bf16, kind="ExternalInput")
    tv = nc.dram_tensor("tv", [P, FTOT], bf16, kind="ExternalInput")
    stats_a = nc.dram_tensor("stats_a", [P, CHUNKS], f32, kind="ExternalOutput")
    stats_q = nc.dram_tensor("stats_q", [1, 512], f32, kind="ExternalOutput")

    with contextlib.ExitStack() as ctx:
        def sbuf(name, shape, dt):
            return ctx.enter_context(nc.sbuf_tensor(name, shape, dt))

        ls_t = sbuf("ls_t", [P, FTOT], bf16)
        mu_t = sbuf("mu_t", [P, FTOT], bf16)
        tv_t = sbuf("tv_t", [P, FTOT], bf16)
        e_t = sbuf("e_t", [P, CF], f32)         # per-chunk scratch (ACT only)
        sp_t = sbuf("sp_t", [P, CF], f32)       # per-chunk scratch (ACT only)
        lv_t = sbuf("lv_t", [P, CF], f32)       # per-chunk scratch (ACT only)
        r_b = [sbuf(f"r_t{i}", [P, CF], bf16) for i in range(2)]
        d_t = sbuf("d_t", [P, CF], bf16)        # per-chunk scratch (DVE only)
        d2_t = sbuf("d2_t", [P, CF], bf16)
        q_b = [sbuf(f"q_t{i}", [P, CF], bf16) for i in range(2)]
        st_a = sbuf("st_a", [P, CHUNKS], f32)
        sq_t = sbuf("sq_t", [1, 512], f32)
        ones_t = sbuf("ones_t", [P, 1], bf16)
        dummy = sbuf("dummy_t", [P, 1], f32)

        psum = ctx.enter_context(nc.psum_tensor("acc", [1, 512], f32))

        sem_ls = [ctx.enter_context(nc.semaphore(f"ls{c}")) for c in range(CHUNKS)]
        sem_mt = [ctx.enter_context(nc.semaphore(f"mt{c}")) for c in range(CHUNKS)]
        sem_act = ctx.enter_context(nc.semaphore("act"))
        sem_dve = ctx.enter_context(nc.semaphore("dve"))
        sem_pe = ctx.enter_context(nc.semaphore("pe"))
        sem_ones = ctx.enter_context(nc.semaphore("ones"))
        sem_out = ctx.enter_context(nc.semaphore("out"))
        block = ctx.enter_context(nc.Block())

        # ACT increments: dummy=1; exp_c=4c+2, ln1_c=4c+3, ln2_c=4c+4,
        # r_c=4c+5 (last ln2 = 16, copy = 18).
        # DVE increments per chunk: sub=3c+1, mul=3c+2, qmul=3c+3.
        # PE increments: 4 per chunk -> 4c+j+1.

        def cs(c):  # chunk slice in the [P, FTOT] tensors
            return slice(c * CF, (c + 1) * CF)

        @block.sync
        def _(sync):
            # ls chunk 0 gates the whole ScalarE chain (the critical path):
            # give it exclusive HBM bandwidth before issuing anything else.
            sync.dma_start(ls_t[:, cs(0)], ls[:, cs(0)]).then_inc(sem_ls[0], 16)
            sync.wait_ge(sem_ls[0], 16)
            for c in range(1, CHUNKS):
                sync.dma_start(ls_t[:, cs(c)], ls[:, cs(c)]).then_inc(sem_ls[c], 16)
            for c in range(CHUNKS):
                sync.dma_start(mu_t[:, cs(c)], mu[:, cs(c)]).then_inc(sem_mt[c], 16)
                sync.dma_start(tv_t[:, cs(c)], tv[:, cs(c)]).then_inc(sem_mt[c], 16)
            sync.wait_ge(sem_act, 16)  # last Ln done -> st_a complete
            sync.dma_start(stats_a[:, :], st_a[:]).then_inc(sem_out, 16)
            sync.wait_ge(sem_act, 18)  # psum->sbuf copy done
            sync.dma_start(stats_q[:, :], sq_t[:]).then_inc(sem_out, 16)

        @block.gpsimd
        def _(gpsimd):
            gpsimd.memset(ones_t[:], 1.0)
            gpsimd.nop().then_inc(sem_ones, 1)

        @block.scalar
        def _(scalar):
            # Prefetch the ln/exp table set during the DMA ramp. scale=0
            # means the (garbage) input is never read: exp(0) = 1.
            scalar.activation(dummy[:], dummy[:], A.Exp, scale=0.0).then_inc(
                sem_act, 1
            )
            for c in range(CHUNKS):
                scalar.wait_ge(sem_ls[c], 16)
                scalar.activation(e_t[:], ls_t[:, cs(c)], A.Exp).then_inc(sem_act, 1)
                scalar.activation(sp_t[:], e_t[:], A.Ln, bias=1.0).then_inc(sem_act, 1)
                scalar.activation(
                    lv_t[:], sp_t[:], A.Ln, accum_out=st_a[:, c : c + 1]
                ).then_inc(sem_act, 1)
                if c >= 2:
                    # r buffer free once chunk c-2's q-mul has read it
                    scalar.wait_ge(sem_dve, 3 * (c - 2) + 3)
                scalar.activation(
                    r_b[c % 2][:], lv_t[:], A.Exp, scale=-1.0
                ).then_inc(sem_act, 1)
            scalar.wait_ge(sem_pe, 4 * CHUNKS)
            scalar.copy(sq_t[:], psum[:]).then_inc(sem_act, 1)

        @block.vector
        def _(vector):
            for c in range(CHUNKS):
                vector.wait_ge(sem_mt[c], 32)
                vector.tensor_sub(d_t[:], tv_t[:, cs(c)], mu_t[:, cs(c)]).then_inc(
                    sem_dve, 1
                )
                vector.tensor_mul(d2_t[:], d_t[:], d_t[:]).then_inc(sem_dve, 1)
                vector.wait_ge(sem_act, 4 * c + 5)  # r_c ready
                if c >= 2:
                    # q buffer free once chunk c-2's matmuls have read it
                    vector.wait_ge(sem_pe, 4 * (c - 2) + 4)
                vector.tensor_mul(q_b[c % 2][:], d2_t[:], r_b[c % 2][:]).then_inc(
                    sem_dve, 1
                )

        @block.tensor
        def _(tensor):
            tensor.wait_ge(sem_ones, 1)
            n = CHUNKS * NMM
            k = 0
            for c in range(CHUNKS):
                tensor.wait_ge(sem_dve, 3 * c + 3)  # q_c ready
                for j in range(NMM):
                    nc.tensor.matmul(
                        psum[:, :],
                        ones_t[:],
                        q_b[c % 2][:, j * 512 : (j + 1) * 512],
                        start=(k == 0),
                        stop=(k == n - 1),
                    ).then_inc(sem_pe, 1)
                    k += 1

    return nc


def _get_program() -> bass.Bass:
    if "nc" not in _prog_cache:
        _prog_cache["nc"] = _build_program()
    return _prog_cache["nc"]


def _pack(x: np.ndarray) -> np.ndarray:
    # [2048, 512] -> [128, 8192]: partition p holds rows p, p+128, ...
    return np.ascontiguousarray(
        x.reshape(RG, P, D).transpose(1, 0, 2).reshape(P, FTOT).astype(BF16)
    )


def kernel(outputs: np.ndarray, targets: np.ndarray, **run_kwargs) -> np.ndarray:
    global last_results
    assert outputs.shape == (B, TWO_D) and targets.shape == (B, TWO_D)

    outputs = np.asarray(outputs, dtype=np.float32)
    targets = np.asarray(targets, dtype=np.float32)

    in_maps = []
    for i in range(N_CORES):
        rows = slice(i * RPC, (i + 1) * RPC)
        in_maps.append(
            {
                "mu": _pack(outputs[rows, :D]),
                "ls": _pack(outputs[rows, D:]),
                "tv": _pack(targets[rows, :D]),
            }
        )

    nc = _get_program()
    res = run_bass_kernel_spmd(nc, in_maps, core_ids=list(range(N_CORES)), **run_kwargs)
    last_results = res

    total = 0.0
    for core_out in res.results:
        total += core_out["stats_a"].astype(np.float64).sum()
        total += core_out["stats_q"].astype(np.float64).sum()

    loss = 0.5 * D * LOG_2PI + 0.5 * total / B
    return np.asarray(loss, dtype=np.float32)


if __name__ == "__main__":
    rng = np.random.default_rng(0)
    o = rng.standard_normal((B, TWO_D), dtype=np.float32)
    t = rng.standard_normal((B, TWO_D), dtype=np.float32)
    got = kernel(o, t)
    m, lsg = o[:, :D].astype(np.float64), o[:, D:].astype(np.float64)
    tvv = t[:, :D].astype(np.float64)
    var = np.log1p(np.exp(lsg))
    want = 0.5 * D * LOG_2PI + 0.5 * np.mean(
        np.sum(np.log(var) + (tvv - m) ** 2 / var, axis=1)
    )
    print("got", got, "want", want, "rel", abs(got - want) / abs(want))
